# revision 26
# baseline (speedup 1.0000x reference)
"""MoE layer (top-2 of 8 experts) on 8 NeuronCores, expert-parallel.

Strategy (per sharding hint): host computes the (tiny) router fp32 matmul +
top-2 dispatch, gathers each expert's tokens, and each core runs that
expert's FFN  relu(X @ W1 + b1) @ W2 + b2, scaled by the renormalized
combine weight, in bf16 on the TensorEngine (fp32 PSUM accumulation).
Host scatter-adds the two expert contributions per token back together.

This does only the top-2 sparse compute (4x less than dense all-expert).
"""

import numpy as np
import ml_dtypes

from concourse import bass, tile
from concourse.bass_utils import run_bass_kernel_spmd

N_CORES = 8
P = 128
D = 1024      # hidden dim
F = 2048      # ffn dim
E = 8         # experts
NCHUNK = 512  # tokens processed per inner chunk (free dim of gemm1)

_BF16 = ml_dtypes.bfloat16


def _split_multi_waits(nc: bass.Bass) -> int:
    """Walrus codegen here allows at most ONE embedded sync-wait per
    instruction; hoist extras into standalone per-engine wait instructions
    placed immediately before (engine queues execute in order, so the
    semantics are identical)."""
    mybir = bass.mybir
    n_split = 0
    for fn in nc.m.functions:
        for blk in fn.blocks:
            out = []
            for inst in blk.instructions:
                si = inst.sync_info
                if si is not None and si.on_wait and len(si.on_wait) > 1:
                    waits = list(si.on_wait)
                    for w in waits[:-1]:
                        ev = mybir.InstEventSemaphore(
                            name=nc.get_next_instruction_name(), ins=[], outs=[])
                        ev.engine = inst.engine
                        ev.sync_info = mybir.SyncInfo(on_wait=[w], on_update=[])
                        out.append(ev)
                        n_split += 1
                    inst.sync_info = mybir.SyncInfo(
                        on_wait=[waits[-1]], on_update=list(si.on_update or []))
                out.append(inst)
            blk.instructions = out
    return n_split


def _build_nc(C: int, b1_is_zero: bool = True, b2_is_zero: bool = True) -> bass.Bass:
    """Per-core kernel: out[C, D] = (relu(X @ W1 + b1) @ W2 + b2) * cw[:, None].

    Inputs (per core / expert):
      xT   [D, C]  bf16   gathered tokens, transposed, zero-padded to C
      w1   [D, F]  bf16
      w2   [F, D]  bf16
      b1t  [P, F/P] f32   b1 tiled so b1t[p, ft] = b1[ft*128 + p]
      b2bc [P, D]  f32    b2 broadcast across partitions
      cwt  [P, C/P] f32   combine weights tiled like b1t (0 on padding)
    """
    dt = bass.mybir.dt
    Act = bass.mybir.ActivationFunctionType
    nc = bass.Bass()

    xT = nc.dram_tensor("xT", [D, C], dt.bfloat16, kind="ExternalInput")
    w1 = nc.dram_tensor("w1", [D, F], dt.bfloat16, kind="ExternalInput")
    w2 = nc.dram_tensor("w2", [F, D], dt.bfloat16, kind="ExternalInput")
    b1t = nc.dram_tensor("b1t", [P, F // P], dt.float32, kind="ExternalInput")
    b2bc = nc.dram_tensor("b2bc", [P, D], dt.float32, kind="ExternalInput")
    cwt = nc.dram_tensor("cwt", [P, C // P], dt.float32, kind="ExternalInput")
    out = nc.dram_tensor("out", [C, D], dt.float32, kind="ExternalOutput")

    KT = D // P   # 8 k-tiles for gemm1
    FT = F // P   # 16 f-tiles (gemm1 out partitions / gemm2 k-tiles)
    ND2 = D // 512  # 2 output column chunks for gemm2

    with tile.TileContext(nc) as tc:
        with (
            tc.tile_pool(name="wres", bufs=1) as wres,
            tc.tile_pool(name="hbuf", bufs=3) as hbuf,
            tc.tile_pool(name="obuf", bufs=4) as obuf,
            tc.tile_pool(name="ps1", bufs=4, space="PSUM") as ps1,
            tc.tile_pool(name="ps2", bufs=4, space="PSUM") as ps2,
        ):
            # Resident weights / constants. Emission order = DMA service
            # order, so stream what the PE needs first: b1, then w1's first
            # column chunk interleaved with x chunk 0, then the rest of w1,
            # then w2/b2/cw (not needed until the first gemm2, ~60us in).
            # PE warmup: dummy matmuls on zeroed SBUF while input DMAs are in
            # flight, so HAM reaches full clock before the first real GEMM.
            warm = wres.tile([P, 512], dt.bfloat16, tag="warm")
            nc.vector.memset(warm[:], 0.0)
            ps_w = ps1.tile([P, 512], dt.float32, tag="ps1")
            for i in range(24):
                nc.tensor.matmul(ps_w[:], warm[:, 0:P], warm[:],
                                 start=(i == 0), stop=(i == 23))

            if not b1_is_zero:
                b1_s = wres.tile([P, F // P], dt.float32, tag="b1")
                nc.sync.dma_start(out=b1_s[:], in_=b1t[:])
            w1_t = [wres.tile([P, F], dt.bfloat16, tag=f"w1_{k}",
                              name=f"w1_{k}")
                    for k in range(KT)]
            # all of x resident; loaded in two waves (fast 512-col start)
            x_all = wres.tile([P, KT, C], dt.bfloat16, tag="x")
            W1C = 512  # w1 column-chunk width of the first DMA wave
            XC0 = min(NCHUNK, C)
            for k in range(KT):
                nc.sync.dma_start(out=w1_t[k][:, 0:W1C],
                                  in_=w1[k * P:(k + 1) * P, 0:W1C])
                nc.sync.dma_start(out=x_all[:, k, 0:XC0],
                                  in_=xT[k * P:(k + 1) * P, 0:XC0])
            for k in range(KT):
                nc.sync.dma_start(
                    out=w1_t[k][:, W1C:F],
                    in_=w1[k * P:(k + 1) * P, W1C:F])
            w2_t = []
            for f in range(FT):
                t = wres.tile([P, D], dt.bfloat16, tag=f"w2_{f}")
                nc.sync.dma_start(out=t[:], in_=w2[f * P:(f + 1) * P, :])
                w2_t.append(t)
            if C > XC0:
                for k in range(KT):
                    nc.sync.dma_start(
                        out=x_all[:, k, XC0:C],
                        in_=xT[k * P:(k + 1) * P, XC0:C])
            if not b2_is_zero:
                b2_s = wres.tile([P, D], dt.float32, tag="b2")
                nc.sync.dma_start(out=b2_s[:], in_=b2bc[:])
            cw_s = wres.tile([P, C // P], dt.float32, tag="cw")
            nc.sync.dma_start(out=cw_s[:], in_=cwt[:])

            for c0 in range(0, C, NCHUNK):
                cn = min(NCHUNK, C - c0)
                # gemm1: hT[f, c] = relu(sum_k w1[k, f] * x[k, c] + b1[f])
                h_t = hbuf.tile([P, FT, NCHUNK], dt.bfloat16, tag="h")
                for f in range(FT):
                    ps = ps1.tile([P, NCHUNK], dt.float32, tag="ps1")
                    for k in range(KT):
                        nc.tensor.matmul(
                            ps[:, :cn],
                            w1_t[k][:, f * P:(f + 1) * P],
                            x_all[:, k, c0:c0 + cn],
                            start=(k == 0),
                            stop=(k == KT - 1),
                        )
                    nc.scalar.activation(
                        h_t[:, f, :cn], ps[:, :cn], Act.Relu,
                        bias=(0.0 if b1_is_zero else b1_s[:, f:f + 1]),
                        scale=1.0,
                    )
                # gemm2: y[c, d] = (sum_f hT[f, c] * w2[f, d] + b2[d]) * cw[c]
                for ct in range(cn // P):
                    ci = c0 // P + ct
                    for d0 in range(ND2):
                        ps_y = ps2.tile([P, 512], dt.float32, tag="ps2")
                        for f in range(FT):
                            nc.tensor.matmul(
                                ps_y[:],
                                h_t[:, f, ct * P:(ct + 1) * P],
                                w2_t[f][:, d0 * 512:(d0 + 1) * 512],
                                start=(f == 0),
                                stop=(f == FT - 1),
                            )
                        o_t = obuf.tile([P, 512], dt.float32, tag="o")
                        if b2_is_zero:
                            # out = psum * cw in one ScalarE op
                            nc.scalar.activation(
                                o_t[:], ps_y[:], Act.Copy,
                                bias=0.0, scale=cw_s[:, ci:ci + 1],
                            )
                        else:
                            nc.vector.tensor_add(
                                o_t[:], ps_y[:], b2_s[:, d0 * 512:(d0 + 1) * 512]
                            )
                            nc.vector.tensor_scalar_mul(
                                o_t[:], o_t[:], cw_s[:, ci:ci + 1]
                            )
                        nc.scalar.dma_start(
                            out=out[ci * P:(ci + 1) * P, d0 * 512:(d0 + 1) * 512],
                            in_=o_t[:],
                        )
    _split_multi_waits(nc)
    return nc


_nc_cache: dict[int, bass.Bass] = {}

# Optional profiling knobs (used by test.py; harness leaves these off).
TRACE = False
TRACE_KWARGS: dict = {}
LAST_RESULTS = None


def kernel(hidden_states, attention_mask, router_w, w1, b1, w2, b2, top_k):
    assert int(top_k) == 2
    hs = np.asarray(hidden_states, np.float32)
    B, S, _ = hs.shape
    T = B * S
    X = hs.reshape(T, D)
    rw = np.asarray(router_w, np.float32)
    w1 = np.asarray(w1, np.float32)
    b1 = np.asarray(b1, np.float32)
    w2 = np.asarray(w2, np.float32)
    b2 = np.asarray(b2, np.float32)

    # --- host router: top-2 of 8, renormalized softmax weights ---
    logits = X @ rw                                    # [T, E] fp32
    m1 = logits.max(-1, keepdims=True)
    i1 = logits.argmax(-1)
    masked = np.where(logits == m1, -np.inf, logits)
    i2 = masked.argmax(-1)
    m2 = masked.max(-1, keepdims=True)
    # softmax denominator cancels in renormalization: w = p_i / (p_1 + p_2)
    e2 = np.exp(m2 - m1)
    c1 = (1.0 / (1.0 + e2)).ravel().astype(np.float32)
    c2 = (e2 / (1.0 + e2)).ravel().astype(np.float32)

    # --- dispatch: gather tokens per expert ---
    # Device capacity is capped at C_CAP tokens per expert (the mean load);
    # the few tokens of over-loaded experts beyond the cap are computed on
    # the host during the combine (keeps all 8 cores' makespan balanced).
    C_CAP = (T * 2 // E) // P * P
    idx_e, cw_e, spill = [], [], []
    for e in range(E):
        sel1 = i1 == e
        sel2 = i2 == e
        idx = np.nonzero(sel1 | sel2)[0]
        cw = np.where(sel1[idx], c1[idx], c2[idx]).astype(np.float32)
        if len(idx) > C_CAP:
            spill.append((e, idx[C_CAP:], cw[C_CAP:]))
            idx, cw = idx[:C_CAP], cw[:C_CAP]
        idx_e.append(idx)
        cw_e.append(cw)
    max_n = max(len(i) for i in idx_e)
    C = max(((max_n + P - 1) // P) * P, NCHUNK)

    b1z = bool(np.all(b1 == 0))
    b2z = bool(np.all(b2 == 0))
    nc = _nc_cache.get((C, b1z, b2z))
    if nc is None:
        nc = _build_nc(C, b1_is_zero=b1z, b2_is_zero=b2z)
        _nc_cache[(C, b1z, b2z)] = nc

    in_maps = []
    for e in range(E):
        idx, cw = idx_e[e], cw_e[e]
        n = len(idx)
        xT = np.zeros((D, C), dtype=_BF16)
        xT[:, :n] = X[idx].T.astype(_BF16)
        cwt = np.zeros((C // P, P), np.float32)
        cwt.ravel()[:n] = cw
        b1t = np.ascontiguousarray(b1[e].reshape(F // P, P).T)
        b2bc = np.broadcast_to(b2[e], (P, D)).copy()
        in_maps.append({
            "xT": xT,
            "w1": np.ascontiguousarray(w1[e]).astype(_BF16),
            "w2": np.ascontiguousarray(w2[e]).astype(_BF16),
            "b1t": b1t,
            "b2bc": b2bc,
            "cwt": np.ascontiguousarray(cwt.T),
        })

    global LAST_RESULTS
    res = run_bass_kernel_spmd(
        nc, in_maps, list(range(N_CORES)), trace=TRACE, **TRACE_KWARGS
    )
    LAST_RESULTS = res

    out_flat = np.zeros((T, D), np.float32)
    for e in range(E):
        idx = idx_e[e]
        out_flat[idx] += res.results[e]["out"][:len(idx)]
    # host-side fp32 compute for capacity-overflow tokens
    for e, idx, cw in spill:
        h = np.maximum(X[idx] @ w1[e] + b1[e], 0.0)
        out_flat[idx] += (h @ w2[e] + b2[e]) * cw[:, None]

    out_flat *= (np.asarray(attention_mask).reshape(T, 1) != 0)
    return out_flat.reshape(B, S, D).astype(np.float32)


# revision 27
# speedup vs baseline: 1.0157x; 1.0157x over previous
"""MoE layer (top-2 of 8 experts) on 8 NeuronCores, expert-parallel.

Strategy (per sharding hint): host computes the (tiny) router fp32 matmul +
top-2 dispatch, gathers each expert's tokens, and each core runs that
expert's FFN  relu(X @ W1 + b1) @ W2 + b2, scaled by the renormalized
combine weight, in bf16 on the TensorEngine (fp32 PSUM accumulation).
Host scatter-adds the two expert contributions per token back together.

This does only the top-2 sparse compute (4x less than dense all-expert).
"""

import numpy as np
import ml_dtypes

from concourse import bass, tile
from concourse.bass_utils import run_bass_kernel_spmd

N_CORES = 8
P = 128
D = 1024      # hidden dim
F = 2048      # ffn dim
E = 8         # experts
NCHUNK = 512  # tokens processed per inner chunk (free dim of gemm1)

_BF16 = ml_dtypes.bfloat16


def _split_multi_waits(nc: bass.Bass) -> int:
    """Walrus codegen here allows at most ONE embedded sync-wait per
    instruction; hoist extras into standalone per-engine wait instructions
    placed immediately before (engine queues execute in order, so the
    semantics are identical)."""
    mybir = bass.mybir
    n_split = 0
    for fn in nc.m.functions:
        for blk in fn.blocks:
            out = []
            for inst in blk.instructions:
                si = inst.sync_info
                if si is not None and si.on_wait and len(si.on_wait) > 1:
                    waits = list(si.on_wait)
                    for w in waits[:-1]:
                        ev = mybir.InstEventSemaphore(
                            name=nc.get_next_instruction_name(), ins=[], outs=[])
                        ev.engine = inst.engine
                        ev.sync_info = mybir.SyncInfo(on_wait=[w], on_update=[])
                        out.append(ev)
                        n_split += 1
                    inst.sync_info = mybir.SyncInfo(
                        on_wait=[waits[-1]], on_update=list(si.on_update or []))
                out.append(inst)
            blk.instructions = out
    return n_split


def _build_nc(C: int, b1_is_zero: bool = True, b2_is_zero: bool = True) -> bass.Bass:
    """Per-core kernel: out[C, D] = (relu(X @ W1 + b1) @ W2 + b2) * cw[:, None].

    Inputs (per core / expert):
      xT   [D, C]  bf16   gathered tokens, transposed, zero-padded to C
      w1   [D, F]  bf16
      w2   [F, D]  bf16
      b1t  [P, F/P] f32   b1 tiled so b1t[p, ft] = b1[ft*128 + p]
      b2bc [P, D]  f32    b2 broadcast across partitions
      cwt  [P, C/P] f32   combine weights tiled like b1t (0 on padding)
    """
    dt = bass.mybir.dt
    Act = bass.mybir.ActivationFunctionType
    nc = bass.Bass()

    xT = nc.dram_tensor("xT", [D, C], dt.bfloat16, kind="ExternalInput")
    w1 = nc.dram_tensor("w1", [D, F], dt.bfloat16, kind="ExternalInput")
    w2 = nc.dram_tensor("w2", [F, D], dt.bfloat16, kind="ExternalInput")
    b1t = nc.dram_tensor("b1t", [P, F // P], dt.float32, kind="ExternalInput")
    b2bc = nc.dram_tensor("b2bc", [P, D], dt.float32, kind="ExternalInput")
    cwt = nc.dram_tensor("cwt", [P, C // P], dt.float32, kind="ExternalInput")
    out = nc.dram_tensor("out", [C, D], dt.float32, kind="ExternalOutput")

    KT = D // P   # 8 k-tiles for gemm1
    FT = F // P   # 16 f-tiles (gemm1 out partitions / gemm2 k-tiles)
    ND2 = D // 512  # 2 output column chunks for gemm2

    with tile.TileContext(nc) as tc:
        with (
            tc.tile_pool(name="wres", bufs=1) as wres,
            tc.tile_pool(name="hbuf", bufs=3) as hbuf,
            tc.tile_pool(name="obuf", bufs=4) as obuf,
            tc.tile_pool(name="ps1", bufs=4, space="PSUM") as ps1,
            tc.tile_pool(name="ps2", bufs=4, space="PSUM") as ps2,
        ):
            # Resident weights / constants. Emission order = DMA service
            # order, so stream what the PE needs first: b1, then w1's first
            # column chunk interleaved with x chunk 0, then the rest of w1,
            # then w2/b2/cw (not needed until the first gemm2, ~60us in).
            # PE warmup: dummy matmuls on zeroed SBUF while input DMAs are in
            # flight, so HAM reaches full clock before the first real GEMM.
            warm = wres.tile([P, 512], dt.bfloat16, tag="warm")
            nc.vector.memset(warm[:], 0.0)
            ps_w = ps1.tile([P, 512], dt.float32, tag="ps1")
            for i in range(10):
                nc.tensor.matmul(ps_w[:], warm[:, 0:P], warm[:],
                                 start=(i == 0), stop=(i == 9))

            if not b1_is_zero:
                b1_s = wres.tile([P, F // P], dt.float32, tag="b1")
                nc.sync.dma_start(out=b1_s[:], in_=b1t[:])
            w1_t = [wres.tile([P, F], dt.bfloat16, tag=f"w1_{k}",
                              name=f"w1_{k}")
                    for k in range(KT)]
            # all of x resident; loaded in two waves (fast 512-col start)
            x_all = wres.tile([P, KT, C], dt.bfloat16, tag="x")
            W1C = 512  # w1 column-chunk width of the first DMA wave
            XC0 = min(NCHUNK, C)
            for k in range(KT):
                nc.sync.dma_start(out=w1_t[k][:, 0:W1C],
                                  in_=w1[k * P:(k + 1) * P, 0:W1C])
                nc.sync.dma_start(out=x_all[:, k, 0:XC0],
                                  in_=xT[k * P:(k + 1) * P, 0:XC0])
            for k in range(KT):
                nc.sync.dma_start(
                    out=w1_t[k][:, W1C:F],
                    in_=w1[k * P:(k + 1) * P, W1C:F])
            w2_t = []
            for f in range(FT):
                t = wres.tile([P, D], dt.bfloat16, tag=f"w2_{f}")
                nc.sync.dma_start(out=t[:], in_=w2[f * P:(f + 1) * P, :])
                w2_t.append(t)
            if C > XC0:
                for k in range(KT):
                    nc.sync.dma_start(
                        out=x_all[:, k, XC0:C],
                        in_=xT[k * P:(k + 1) * P, XC0:C])
            if not b2_is_zero:
                b2_s = wres.tile([P, D], dt.float32, tag="b2")
                nc.sync.dma_start(out=b2_s[:], in_=b2bc[:])
            cw_s = wres.tile([P, C // P], dt.float32, tag="cw")
            nc.sync.dma_start(out=cw_s[:], in_=cwt[:])

            for c0 in range(0, C, NCHUNK):
                cn = min(NCHUNK, C - c0)
                # gemm1: hT[f, c] = relu(sum_k w1[k, f] * x[k, c] + b1[f])
                h_t = hbuf.tile([P, FT, NCHUNK], dt.bfloat16, tag="h")
                for f in range(FT):
                    ps = ps1.tile([P, NCHUNK], dt.float32, tag="ps1")
                    for k in range(KT):
                        nc.tensor.matmul(
                            ps[:, :cn],
                            w1_t[k][:, f * P:(f + 1) * P],
                            x_all[:, k, c0:c0 + cn],
                            start=(k == 0),
                            stop=(k == KT - 1),
                        )
                    nc.scalar.activation(
                        h_t[:, f, :cn], ps[:, :cn], Act.Relu,
                        bias=(0.0 if b1_is_zero else b1_s[:, f:f + 1]),
                        scale=1.0,
                    )
                # gemm2: y[c, d] = (sum_f hT[f, c] * w2[f, d] + b2[d]) * cw[c]
                for ct in range(cn // P):
                    ci = c0 // P + ct
                    for d0 in range(ND2):
                        ps_y = ps2.tile([P, 512], dt.float32, tag="ps2")
                        for f in range(FT):
                            nc.tensor.matmul(
                                ps_y[:],
                                h_t[:, f, ct * P:(ct + 1) * P],
                                w2_t[f][:, d0 * 512:(d0 + 1) * 512],
                                start=(f == 0),
                                stop=(f == FT - 1),
                            )
                        o_t = obuf.tile([P, 512], dt.float32, tag="o")
                        if b2_is_zero:
                            # out = psum * cw in one ScalarE op
                            nc.scalar.activation(
                                o_t[:], ps_y[:], Act.Copy,
                                bias=0.0, scale=cw_s[:, ci:ci + 1],
                            )
                        else:
                            nc.vector.tensor_add(
                                o_t[:], ps_y[:], b2_s[:, d0 * 512:(d0 + 1) * 512]
                            )
                            nc.vector.tensor_scalar_mul(
                                o_t[:], o_t[:], cw_s[:, ci:ci + 1]
                            )
                        nc.scalar.dma_start(
                            out=out[ci * P:(ci + 1) * P, d0 * 512:(d0 + 1) * 512],
                            in_=o_t[:],
                        )
    _split_multi_waits(nc)
    return nc


_nc_cache: dict[int, bass.Bass] = {}

# Optional profiling knobs (used by test.py; harness leaves these off).
TRACE = False
TRACE_KWARGS: dict = {}
LAST_RESULTS = None


def kernel(hidden_states, attention_mask, router_w, w1, b1, w2, b2, top_k):
    assert int(top_k) == 2
    hs = np.asarray(hidden_states, np.float32)
    B, S, _ = hs.shape
    T = B * S
    X = hs.reshape(T, D)
    rw = np.asarray(router_w, np.float32)
    w1 = np.asarray(w1, np.float32)
    b1 = np.asarray(b1, np.float32)
    w2 = np.asarray(w2, np.float32)
    b2 = np.asarray(b2, np.float32)

    # --- host router: top-2 of 8, renormalized softmax weights ---
    logits = X @ rw                                    # [T, E] fp32
    m1 = logits.max(-1, keepdims=True)
    i1 = logits.argmax(-1)
    masked = np.where(logits == m1, -np.inf, logits)
    i2 = masked.argmax(-1)
    m2 = masked.max(-1, keepdims=True)
    # softmax denominator cancels in renormalization: w = p_i / (p_1 + p_2)
    e2 = np.exp(m2 - m1)
    c1 = (1.0 / (1.0 + e2)).ravel().astype(np.float32)
    c2 = (e2 / (1.0 + e2)).ravel().astype(np.float32)

    # --- dispatch: gather tokens per expert ---
    # Device capacity is capped at C_CAP tokens per expert (the mean load);
    # the few tokens of over-loaded experts beyond the cap are computed on
    # the host during the combine (keeps all 8 cores' makespan balanced).
    C_CAP = (T * 2 // E) // P * P
    idx_e, cw_e, spill = [], [], []
    for e in range(E):
        sel1 = i1 == e
        sel2 = i2 == e
        idx = np.nonzero(sel1 | sel2)[0]
        cw = np.where(sel1[idx], c1[idx], c2[idx]).astype(np.float32)
        if len(idx) > C_CAP:
            spill.append((e, idx[C_CAP:], cw[C_CAP:]))
            idx, cw = idx[:C_CAP], cw[:C_CAP]
        idx_e.append(idx)
        cw_e.append(cw)
    max_n = max(len(i) for i in idx_e)
    C = max(((max_n + P - 1) // P) * P, NCHUNK)

    b1z = bool(np.all(b1 == 0))
    b2z = bool(np.all(b2 == 0))
    nc = _nc_cache.get((C, b1z, b2z))
    if nc is None:
        nc = _build_nc(C, b1_is_zero=b1z, b2_is_zero=b2z)
        _nc_cache[(C, b1z, b2z)] = nc

    in_maps = []
    for e in range(E):
        idx, cw = idx_e[e], cw_e[e]
        n = len(idx)
        xT = np.zeros((D, C), dtype=_BF16)
        xT[:, :n] = X[idx].T.astype(_BF16)
        cwt = np.zeros((C // P, P), np.float32)
        cwt.ravel()[:n] = cw
        b1t = np.ascontiguousarray(b1[e].reshape(F // P, P).T)
        b2bc = np.broadcast_to(b2[e], (P, D)).copy()
        in_maps.append({
            "xT": xT,
            "w1": np.ascontiguousarray(w1[e]).astype(_BF16),
            "w2": np.ascontiguousarray(w2[e]).astype(_BF16),
            "b1t": b1t,
            "b2bc": b2bc,
            "cwt": np.ascontiguousarray(cwt.T),
        })

    global LAST_RESULTS
    res = run_bass_kernel_spmd(
        nc, in_maps, list(range(N_CORES)), trace=TRACE, **TRACE_KWARGS
    )
    LAST_RESULTS = res

    out_flat = np.zeros((T, D), np.float32)
    for e in range(E):
        idx = idx_e[e]
        out_flat[idx] += res.results[e]["out"][:len(idx)]
    # host-side fp32 compute for capacity-overflow tokens
    for e, idx, cw in spill:
        h = np.maximum(X[idx] @ w1[e] + b1[e], 0.0)
        out_flat[idx] += (h @ w2[e] + b2[e]) * cw[:, None]

    out_flat *= (np.asarray(attention_mask).reshape(T, 1) != 0)
    return out_flat.reshape(B, S, D).astype(np.float32)
